# revision 22
# baseline (speedup 1.0000x reference)
"""Trainium2 Bass kernel for nn_Att_0_layer2 (sparse_attention).

Math (per (b, n) pair):
  v = att1 @ obj_reps                      # [A,O]@[O,D] -- never materialized:
  vq@W1 = v@W1v + q@W1q  ==>  att1 @ (obj_reps @ W1v) + (q @ W1q)
  jointT = relu(objW.T @ att1.T + bias)    # [H, A], objW = obj@W1v, bias = q@W1q + b1
  logits = jointT.T @ W2 (/t folded into W2 host-side; b2 dropped: softmax-invariant)
  att2 = softmax(logits masked by tags>0)
  out = att2 @ att1                        # [O]

Sparsity: tokens with tag==0 get -1e30 logits -> softmax weight 0 -> they
contribute NOTHING downstream.  The mask is host-visible, so att1 is
compacted to the ~A/2 surviving rows (padded to a multiple of 128; pad
slots carry a -1e30 additive mask so they exp to 0).  Cuts DMA (the
bottleneck) and all per-pair compute ~0.6x.

Sharding: pure data parallel, B=64 split 8 ways (8 b's per core).
Device, per pair group (2 pairs per att1 DMA):
  PE:  jointT chunks = objW[b].T @ att1T   (2 matmuls)
       logits: lhsT=jointT chunk, rhs=W2   (AC matmuls, free-size 1)
       final:  lhsT=att1 natural chunk, rhs=exp col -> ps_out[:, i]
               (AC matmuls, free-size 1; raw exp, normalized on host)
  ACT: relu chunks 0..AC-3 (bias col i); exp of both pairs' masked logits
  DVE: relu last 2 chunks, mask add, S-reduce -> outbuf[:, NP+i]
Batched at setup: bias_all (all 32 pairs), objW (all 8 b's, from
host-pretransposed obj).  Output: one [128, 2*NP] DMA of raw out columns +
exp-sums; host does out[i,:] = cols[:,i] / sum(s[:,i]).
"""

import sys
import os
import numpy as np

sys.path.insert(0, "/opt/trn_rl_repo")

B, N, A, O, D, Q, H = 64, 4, 1024, 128, 256, 256, 128
NCORES = 8
BPC = B // NCORES   # batches per core
P = 128             # partitions
NP = BPC * N        # pairs per core (32)
NG = NP // 2        # pair groups (2 pairs per group)

TRACE = False
TRACE_KW = {}

_NC_CACHE = {}
_NC_LAST = None


def _build_nc(AC):
    """AC = number of 128-token chunks per pair after compaction."""
    import concourse.bacc as bacc
    import concourse.mybir as mybir
    from concourse.tile import TileContext

    f32 = mybir.dt.float32
    bf16 = mybir.dt.bfloat16
    AF = mybir.ActivationFunctionType
    OP = mybir.AluOpType
    AX = mybir.AxisListType

    ACU = AC * P
    ACT_C = max(1, AC - 2)  # relu chunks on ACT (from ps_ja)
    DVE_C = AC - ACT_C      # relu chunks on DVE (from ps_jb)

    nc = bacc.Bacc("TRN2", target_bir_lowering=False)

    # att1 per pair: [natural [a_in, AC, O] | transposed [o, ACU]] bf16
    att1_d = nc.declare_dram_parameter("att1", [BPC, N, P, 2 * ACU], bf16,
                                       isOutput=False)
    # objW = obj @ W1v precomputed on host: [o, b, h]
    objw_d = nc.declare_dram_parameter("objw", [P, BPC, H], bf16,
                                       isOutput=False)
    # q pre-transposed on host to [qc, q_in, pair]
    q_d = nc.declare_dram_parameter("q", [2, P, NP], bf16, isOutput=False)
    # additive mask (0 real / -1e30 pad), host layout [a_in, b, n, c]
    negm_d = nc.declare_dram_parameter("negm", [P, BPC, N, AC], f32,
                                       isOutput=False)
    # W1q half only (bias path), host layout [q_in, qc, h]
    w1_d = nc.declare_dram_parameter("w1", [P, 2, H], bf16, isOutput=False)
    w2_d = nc.declare_dram_parameter("w2", [H, 1], bf16, isOutput=False)
    b1_d = nc.declare_dram_parameter("b1", [H, 1], f32, isOutput=False)
    # raw output columns [o, pair] and exp-sums [a_in, pair]
    outs_d = nc.declare_dram_parameter("outs", [P, 2 * NP], f32, isOutput=True)

    with TileContext(nc) as tc:
        with (
            tc.tile_pool(name="const", bufs=1) as constp,
            tc.tile_pool(name="att1b", bufs=7) as att1b_p,
            tc.tile_pool(name="joint", bufs=3) as joint_p,
            tc.tile_pool(name="small", bufs=4) as small_p,
            tc.tile_pool(name="psja", bufs=3, space="PSUM") as psja_p,
            tc.tile_pool(name="psjb", bufs=2, space="PSUM") as psjb_p,
            tc.tile_pool(name="psl", bufs=2, space="PSUM") as psl_p,
            tc.tile_pool(name="pso", bufs=1, space="PSUM") as pso_p,
        ):
            # persistent accumulators / output staging
            ps_out = pso_p.tile([P, NP], f32)          # [o, pair]
            outbuf = constp.tile([P, 2 * NP], f32)     # [:, :NP]=cols, [NP:]=s

            def load_att1(g):
                t = att1b_p.tile([P, 2, 2, AC, O], bf16, tag="a1c")
                b, n = divmod(2 * g, N)
                nc.sync.dma_start(
                    t, att1_d[b, n:n + 2].rearrange(
                        "n p (t c a) -> p n t c a", t=2, c=AC))
                return t

            def emit_setup():
                w1_b = constp.tile([P, 2, H], bf16)
                nc.sync.dma_start(w1_b, w1_d[:])

                objW = constp.tile([P, BPC, H], bf16)
                nc.sync.dma_start(objW, objw_d[:])

                q_all_b = constp.tile([P, 2, NP], bf16)
                nc.sync.dma_start(q_all_b, q_d[:].rearrange("c p m -> p c m"))

                negm = constp.tile([P, BPC, N, AC], f32)
                nc.sync.dma_start(negm, negm_d[:])

                w2_b = constp.tile([H, 1], bf16)
                nc.sync.dma_start(w2_b, w2_d[:])
                b1_sb = constp.tile([H, 1], f32)
                nc.sync.dma_start(b1_sb, b1_d[:])

                # bias_all[:, i] = W1q.T @ q_i + b1 for all pairs: [H, NP]
                ps_bias = psja_p.tile([H, NP], f32, tag="ja")
                for c in range(2):
                    nc.tensor.matmul(ps_bias, w1_b[:, c, :],
                                     q_all_b[:, c, :],
                                     start=(c == 0), stop=(c == 1))
                bias_all = constp.tile([H, NP], f32)
                nc.vector.tensor_scalar(bias_all, ps_bias, b1_sb[:, 0:1],
                                        None, OP.add)
                return w2_b, objW, bias_all, negm

            def joint_mm(i, b, objW, att1_c, j):
                ps_ja = psja_p.tile([H, ACT_C * P], f32, tag="ja")
                nc.tensor.matmul(ps_ja, objW[:, b, :],
                                 att1_c[:, j, 1, 0:ACT_C, :],
                                 start=True, stop=True)
                ps_jb = psjb_p.tile([H, DVE_C * P], f32, tag="jb")
                nc.tensor.matmul(ps_jb, objW[:, b, :],
                                 att1_c[:, j, 1, ACT_C:AC, :],
                                 start=True, stop=True)
                return ps_ja, ps_jb

            def relu_act(i, bias_all, ps_ja, jointT):
                nc.scalar.activation(
                    jointT[:, 0:ACT_C, :],
                    ps_ja[:].rearrange("p (c a) -> p c a", c=ACT_C),
                    AF.Relu, bias=bias_all[:, i:i + 1])

            def relu_dve(i, bias_all, ps_jb, jointT):
                nc.vector.tensor_scalar(
                    jointT[:, ACT_C:AC, :],
                    ps_jb[:].rearrange("p (c a) -> p c a", c=DVE_C),
                    bias_all[:, i:i + 1], 0.0, OP.add, OP.max)

            def logits_mm(jointT, w2_b):
                ps_l = psl_p.tile([P, AC], f32, tag="psl")
                for c in range(AC):
                    nc.tensor.matmul(ps_l[:, c:c + 1], jointT[:, c, :], w2_b,
                                     start=True, stop=True)
                return ps_l

            def mask_add(b, n, negm, ps_l, masked2, j):
                nc.vector.tensor_tensor(masked2[:, j, :], ps_l,
                                        negm[:, b, n, :], OP.add)

            def exp_reduce(g, masked2):
                e2 = small_p.tile([P, 2, AC], bf16, tag="e2")
                nc.scalar.activation(e2, masked2, AF.Exp)
                i0 = 2 * g
                nc.vector.tensor_reduce(outbuf[:, NP + i0:NP + i0 + 2], e2,
                                        AX.X, OP.add)
                return e2

            def final_mm(g, att1_c, e2):
                for j in range(2):
                    i = 2 * g + j
                    for c in range(AC):
                        nc.tensor.matmul(ps_out[:, i:i + 1],
                                         att1_c[:, j, 0, c, :],
                                         e2[:, j, c:c + 1],
                                         start=(c == 0), stop=(c == AC - 1))

            # ---- emission ----
            LOOKAHEAD = 3                      # groups prefetched ahead
            ELAG = 2                           # exp/final lag in groups
            w2_b, objW, bias_all, negm = emit_setup()
            loads = {g: load_att1(g) for g in range(LOOKAHEAD)}

            pend_ab = {}    # g -> (att1_c, masked2)
            for g in range(NG):
                if g + LOOKAHEAD < NG:
                    loads[g + LOOKAHEAD] = load_att1(g + LOOKAHEAD)
                att1_c = loads.pop(g)
                i0 = 2 * g
                b0, n0 = divmod(i0, N)
                b1_, n1 = divmod(i0 + 1, N)
                jointT0 = joint_p.tile([H, AC, P], bf16, tag="joint")
                jointT1 = joint_p.tile([H, AC, P], bf16, tag="joint")
                masked2 = small_p.tile([P, 2, AC], f32, tag="mask")

                # lagged exp/reduce first: its deps finished a full period
                # ago, so the in-order ACT queue never stalls on it
                if g >= ELAG:
                    e2p = exp_reduce(g - ELAG, pend_ab[g - ELAG][1])
                ja0, jb0 = joint_mm(i0, b0, objW, att1_c, 0)
                relu_act(i0, bias_all, ja0, jointT0)
                relu_dve(i0, bias_all, jb0, jointT0)
                ja1, jb1 = joint_mm(i0 + 1, b1_, objW, att1_c, 1)
                relu_act(i0 + 1, bias_all, ja1, jointT1)
                relu_dve(i0 + 1, bias_all, jb1, jointT1)
                if g >= ELAG:
                    final_mm(g - ELAG, pend_ab.pop(g - ELAG)[0], e2p)
                l0 = logits_mm(jointT0, w2_b)
                mask_add(b0, n0, negm, l0, masked2, 0)
                l1 = logits_mm(jointT1, w2_b)
                mask_add(b1_, n1, negm, l1, masked2, 1)
                pend_ab[g] = (att1_c, masked2)

            for g in range(max(0, NG - ELAG), NG):
                e2p = exp_reduce(g, pend_ab[g][1])
                final_mm(g, pend_ab.pop(g)[0], e2p)

            nc.vector.tensor_copy(outbuf[:, 0:NP], ps_out)
            nc.sync.dma_start(outs_d[:], outbuf)

    nc.compile()
    return nc


def _get_nc(AC=None):
    global _NC_LAST
    if AC is None:
        if _NC_LAST is not None:
            return _NC_LAST
        AC = 5
    if AC not in _NC_CACHE:
        _NC_CACHE[AC] = _build_nc(AC)
    _NC_LAST = _NC_CACHE[AC]
    return _NC_LAST


def kernel(**inputs):
    q = np.asarray(inputs["q"], dtype=np.float32)
    att1 = np.asarray(inputs["att1"], dtype=np.float32)
    obj = np.asarray(inputs["obj_reps"], dtype=np.float32)
    tags = np.asarray(inputs["tags_attention"], dtype=np.int32)
    W1 = np.asarray(inputs["W1"], dtype=np.float32)
    b1 = np.asarray(inputs["b1"], dtype=np.float32)
    W2 = np.asarray(inputs["W2"], dtype=np.float32)
    t = float(np.asarray(inputs["t"]))
    # b2 dropped: constant shift is softmax-invariant.

    import ml_dtypes

    # ---- sparsity compaction: keep only tag==1 rows of att1 ----
    cnt = tags.sum(axis=-1)                      # [B, N]
    AC = max(2, int(-(-int(cnt.max()) // P)))    # chunks of 128
    ACU = AC * P
    order = np.argsort(1 - tags, axis=-1, kind="stable")[..., :ACU]  # [B,N,ACU]
    att1_comp = np.take_along_axis(att1, order[..., None], axis=2)   # [B,N,ACU,O]
    valid = np.take_along_axis(tags, order, axis=2)                  # [B,N,ACU]
    negm_full = (valid.astype(np.float32) - 1.0) * 1e30              # 0 / -1e30

    att1_bf = att1_comp.astype(ml_dtypes.bfloat16)
    nat = att1_bf.reshape(B, N, AC, P, O).transpose(0, 1, 3, 2, 4) \
        .reshape(B, N, P, ACU)
    trans = att1_bf.transpose(0, 1, 3, 2)                            # [B,N,O,ACU]
    att1_c = np.concatenate([nat, trans], axis=-1)                   # [B,N,128,2ACU]

    nc = _get_nc(AC)
    from concourse.bass_utils import run_bass_kernel_spmd

    w1r = np.ascontiguousarray(
        W1[D:].reshape(2, P, H).transpose(1, 0, 2)).astype(ml_dtypes.bfloat16)
    w2s = np.ascontiguousarray((W2 / t).reshape(H, 1)).astype(ml_dtypes.bfloat16)
    b1r = np.ascontiguousarray(b1.reshape(H, 1))
    # objW = obj @ W1v on host: [B, O, H]
    objw = (obj.reshape(B * O, D) @ W1[:D]).reshape(B, O, H) \
        .astype(ml_dtypes.bfloat16)

    in_maps = []
    for k in range(NCORES):
        bs = slice(k * BPC, (k + 1) * BPC)
        q_t = q[bs].reshape(BPC * N, 2, P).transpose(1, 2, 0) \
            .astype(ml_dtypes.bfloat16)                              # [2,P,NP]
        objw_t = objw[bs].transpose(1, 0, 2)                         # [P,BPC,H]
        negm_t = negm_full[bs].reshape(BPC, N, AC, P).transpose(3, 0, 1, 2)
        in_maps.append({
            "att1": np.ascontiguousarray(att1_c[bs]),
            "objw": np.ascontiguousarray(objw_t),
            "q": np.ascontiguousarray(q_t),
            "negm": np.ascontiguousarray(negm_t),
            "w1": w1r,
            "w2": w2s,
            "b1": b1r,
        })

    res = run_bass_kernel_spmd(nc, in_maps, core_ids=list(range(NCORES)),
                               trace=TRACE, **TRACE_KW)
    outs = []
    for r in res.results:
        raw = r["outs"]                          # [P, 2*NP] f32
        cols = raw[:, :NP]                       # [o, pair]
        s = raw[:, NP:].sum(axis=0)              # [pair]
        outs.append((cols / s[None, :]).T.reshape(BPC, N, O))
    out = np.concatenate(outs, axis=0)
    if TRACE:
        print("HW exec time:", res.exec_time_ns, "ns",
              "(mean:", res.mean_exec_time_ns, ")")
        if res.instructions_and_trace:
            print("trace:", res.instructions_and_trace[1])
    return out
